# revision 5
# baseline (speedup 1.0000x reference)
"""Position-attention kernel for Trainium2 (8 NeuronCores, SPMD).

Reference computation (per batch b):
    q = Wq @ x + bq        [32, 4096]
    k = Wk @ x + bk        [32, 4096]
    v = Wv @ x + bv        [256, 4096]
    attn = softmax_j(q_i . k_j)           [4096, 4096]
    out[c, i] = sum_j v[c, j] attn[i, j]
    y = gamma * out + x

Sharding: B=4 batches x 2 query-halves -> 8 cores. Each core computes the
full softmax rows for its 2048 queries against all 4096 keys of its batch.
Host rotates x columns per core so the core's query half is always columns
0:2048 (softmax and the PV contraction are invariant to key/value column
order, as long as K and V use the same order).

Device-side layout tricks:
  - scores computed transposed (sT[j, i]) so that exp output tiles feed the
    PV matmul as the stationary operand without any transposes.
  - V projected directly in transposed layout vT[j, c] (lhsT = x block),
    with an extra ones-column so the PV matmul also produces the softmax
    denominator sum_j exp as output column 256 (per-partition = per-query).
  - epilogue is a single fused per-partition op:
        y_T[i, c] = (out[i, c] * gamma/sum_i) + x_T[i, c]
    written transposed; the host transposes back (pure layout).
"""

import os
import numpy as np

P = 128
B = 4
C = 256
CQ = 32
H = W = 64
N = H * W            # 4096 keys per batch
NH = N // 2          # 2048 queries per core
NCB = C // P         # 2 channel blocks
ST = 512             # query supertile
NST = NH // ST       # 4
JB = N // P          # 32 key blocks

_PROG = None         # cached (nc,) build
LAST_RESULT = None   # BassKernelResults of the last run (for test harness)


def _build_program():
    import concourse.mybir as mybir
    import concourse.tile as tile
    from concourse import bacc
    from concourse.bass import ds

    fp32 = mybir.dt.float32
    bf16 = mybir.dt.bfloat16

    nc = bacc.Bacc(None, target_bir_lowering=False, debug=False)

    x_d = nc.declare_dram_parameter("x", [C, N], fp32, isOutput=False)
    xqT_d = nc.declare_dram_parameter("xqT", [NH, C], fp32, isOutput=False)
    wq_d = nc.declare_dram_parameter("wqT", [C, CQ], fp32, isOutput=False)
    wk_d = nc.declare_dram_parameter("wkT", [C, CQ], fp32, isOutput=False)
    wv_d = nc.declare_dram_parameter("wvT", [C, C], fp32, isOutput=False)
    bq_d = nc.declare_dram_parameter("bq", [CQ, 1], fp32, isOutput=False)
    bk_d = nc.declare_dram_parameter("bk", [CQ, 1], fp32, isOutput=False)
    bv_d = nc.declare_dram_parameter("bv_bc", [P, C], fp32, isOutput=False)
    gm_d = nc.declare_dram_parameter("gamma_bc", [P, 1], fp32, isOutput=False)
    yT_d = nc.declare_dram_parameter("yT", [NH, C], fp32, isOutput=True)

    with tile.TileContext(nc) as tc:
        with (
            tc.tile_pool(name="singles", bufs=1) as singles,
            tc.tile_pool(name="epool", bufs=3) as epool,
            tc.tile_pool(name="stpool", bufs=4) as stpool,
            tc.tile_pool(name="ivpool", bufs=4) as ivpool,
            tc.tile_pool(name="pp_mm", bufs=2, space="PSUM") as pp_mm,
            tc.tile_pool(name="pp_out", bufs=4, space="PSUM") as pp_out,
        ):
            # ---- persistent SBUF tensors ----
            x_sb = singles.tile([P, NCB, N], fp32)
            xqT_sb = singles.tile([P, NH // P, C], fp32)
            wq_sb = singles.tile([P, NCB, CQ], fp32)
            wk_sb = singles.tile([P, NCB, CQ], fp32)
            wv_sb = singles.tile([P, NCB, C], fp32)
            bq_sb = singles.tile([CQ, 1], fp32)
            bk_sb = singles.tile([CQ, 1], fp32)
            bv_sb = singles.tile([P, C], fp32)
            gm_sb = singles.tile([P, 1], fp32)
            kf_sb = singles.tile([P, N], bf16)       # rows 32.. stay zero
            q_sb = singles.tile([P, NH], bf16)       # rows 32.. stay zero
            vT_sb = singles.tile([P, JB, C + 1], bf16)  # col C is all-ones

            # ---- input DMAs (split x into chunks to spread DMA queues) ----
            for cb in range(NCB):
                for hc in range(4):
                    sl = ds(hc * 1024, 1024)
                    nc.sync.dma_start(
                        out=x_sb[:, cb, sl], in_=x_d[cb * P:(cb + 1) * P, sl]
                    )
            nc.sync.dma_start(
                out=xqT_sb[:], in_=xqT_d.rearrange("(o p) c -> p o c", p=P)
            )
            nc.sync.dma_start(out=wq_sb[:], in_=wq_d.rearrange("(o p) m -> p o m", p=P))
            nc.sync.dma_start(out=wk_sb[:], in_=wk_d.rearrange("(o p) m -> p o m", p=P))
            nc.sync.dma_start(out=wv_sb[:], in_=wv_d.rearrange("(o p) m -> p o m", p=P))
            nc.sync.dma_start(out=bq_sb[:], in_=bq_d[:])
            nc.sync.dma_start(out=bk_sb[:], in_=bk_d[:])
            nc.sync.dma_start(out=bv_sb[:], in_=bv_d[:])
            nc.sync.dma_start(out=gm_sb[:], in_=gm_d[:])

            # zero the padding rows (contraction runs over all 128 partitions)
            nc.vector.memset(kf_sb[:], 0.0)
            nc.vector.memset(q_sb[:], 0.0)
            nc.vector.memset(vT_sb[:, :, C:C + 1], 1.0)

            # ---- K projection: kf[d, j] = Wk @ x + bk  (bf16, d on partitions)
            for t in range(N // ST):
                ps = pp_mm.tile([CQ, ST], fp32, tag="mm")
                nc.tensor.matmul(
                    ps, wk_sb[:, 0], x_sb[:, 0, ds(t * ST, ST)], start=True, stop=False
                )
                nc.tensor.matmul(
                    ps, wk_sb[:, 1], x_sb[:, 1, ds(t * ST, ST)], start=False, stop=True
                )
                nc.vector.tensor_scalar_add(kf_sb[0:CQ, ds(t * ST, ST)], ps, bk_sb)

            # ---- Q projection (only our half: columns 0:NH) ----
            for t in range(NH // ST):
                ps = pp_mm.tile([CQ, ST], fp32, tag="mm")
                nc.tensor.matmul(
                    ps, wq_sb[:, 0], x_sb[:, 0, ds(t * ST, ST)], start=True, stop=False
                )
                nc.tensor.matmul(
                    ps, wq_sb[:, 1], x_sb[:, 1, ds(t * ST, ST)], start=False, stop=True
                )
                nc.vector.tensor_scalar_add(q_sb[0:CQ, ds(t * ST, ST)], ps, bq_sb)

            # ---- V projection, transposed: vT[j, c] = sum_c' x[c', j] WvT[c', c]
            for j in range(JB):
                ps = pp_mm.tile([P, C], fp32, tag="mm")
                nc.tensor.matmul(
                    ps, x_sb[:, 0, ds(j * P, P)], wv_sb[:, 0], start=True, stop=False
                )
                nc.tensor.matmul(
                    ps, x_sb[:, 1, ds(j * P, P)], wv_sb[:, 1], start=False, stop=True
                )
                nc.vector.tensor_add(vT_sb[:, j, 0:C], ps, bv_sb)

            # ---- attention over query supertiles ----
            for st_i in range(NST):
                q_sl = q_sb[:, ds(st_i * ST, ST)]
                outs = [
                    pp_out.tile([P, C + 1], fp32, tag="out", name=f"out_{st_i}_{k}")
                    for k in range(4)
                ]
                for g in range(JB // 2):
                    sps = pp_mm.tile([P, 2, ST], fp32, tag="mm")
                    for u in range(2):
                        j = 2 * g + u
                        nc.tensor.matmul(
                            sps[:, u], kf_sb[:, ds(j * P, P)], q_sl,
                            start=True, stop=True,
                        )
                    e = epool.tile([P, 2, ST], bf16)
                    nc.scalar.activation(
                        e, sps, mybir.ActivationFunctionType.Exp
                    )
                    for u in range(2):
                        j = 2 * g + u
                        for ib in range(4):
                            nc.tensor.matmul(
                                outs[ib],
                                e[:, u, ds(ib * P, P)],
                                vT_sb[:, j, :],
                                start=(j == 0), stop=(j == JB - 1),
                            )
                # epilogue: per-partition normalize + gamma + residual, store
                for ib in range(4):
                    row = st_i * 4 + ib
                    inv = ivpool.tile([P, 1], fp32)
                    nc.vector.reciprocal(inv, outs[ib][:, C:C + 1])
                    nc.vector.tensor_scalar_mul(inv, inv, gm_sb)
                    stg = stpool.tile([P, C], fp32)
                    nc.vector.scalar_tensor_tensor(
                        stg, outs[ib][:, 0:C], inv, xqT_sb[:, row, :],
                        op0=mybir.AluOpType.mult,
                        op1=mybir.AluOpType.add,
                    )
                    nc.sync.dma_start(out=yT_d[ds(row * P, P), :], in_=stg)

    return nc


def _get_program():
    global _PROG
    if _PROG is None:
        _PROG = _build_program()
        if not _PROG.is_finalized():
            _PROG.finalize()
    return _PROG


def kernel(x, Wq, bq, Wk, bk, Wv, bv, gamma):
    global LAST_RESULT
    from concourse.bass_utils import run_bass_kernel_spmd

    x = np.ascontiguousarray(np.asarray(x, dtype=np.float32))
    Wq = np.asarray(Wq, dtype=np.float32)
    bq = np.asarray(bq, dtype=np.float32)
    Wk = np.asarray(Wk, dtype=np.float32)
    bk = np.asarray(bk, dtype=np.float32)
    Wv = np.asarray(Wv, dtype=np.float32)
    bv = np.asarray(bv, dtype=np.float32)
    gamma = np.asarray(gamma, dtype=np.float32)

    wqT = np.ascontiguousarray(Wq.T)
    wkT = np.ascontiguousarray(Wk.T)
    wvT = np.ascontiguousarray(Wv.T)
    bq2 = np.ascontiguousarray(bq[:, None])
    bk2 = np.ascontiguousarray(bk[:, None])
    bv_bc = np.ascontiguousarray(np.broadcast_to(bv[None, :], (P, C)))
    gm_bc = np.full((P, 1), float(gamma.reshape(-1)[0]), dtype=np.float32)

    xf = x.reshape(B, C, N)
    in_maps = []
    for core in range(8):
        b, h = core // 2, core % 2
        xb = xf[b]
        if h == 0:
            x_roll = np.ascontiguousarray(xb)
        else:
            x_roll = np.ascontiguousarray(
                np.concatenate([xb[:, NH:], xb[:, :NH]], axis=1)
            )
        xqT = np.ascontiguousarray(x_roll[:, :NH].T)
        in_maps.append({
            "x": x_roll,
            "xqT": xqT,
            "wqT": wqT,
            "wkT": wkT,
            "wvT": wvT,
            "bq": bq2,
            "bk": bk2,
            "bv_bc": bv_bc,
            "gamma_bc": gm_bc,
        })

    nc = _get_program()
    res = run_bass_kernel_spmd(
        nc, in_maps, core_ids=list(range(8)),
        trace=bool(os.environ.get("BASS_TRACE")),
    )
    LAST_RESULT = res

    out = np.empty((B, C, N), dtype=np.float32)
    for core in range(8):
        b, h = core // 2, core % 2
        yT = res.results[core]["yT"]
        out[b][:, h * NH:(h + 1) * NH] = yT.T
    return out.reshape(B, C, H, W)


# revision 8
# speedup vs baseline: 1.3170x; 1.3170x over previous
"""Position-attention kernel for Trainium2 (8 NeuronCores, SPMD).

Reference computation (per batch b):
    q = Wq @ x + bq        [32, 4096]
    k = Wk @ x + bk        [32, 4096]
    v = Wv @ x + bv        [256, 4096]
    attn = softmax_j(q_i . k_j)           [4096, 4096]
    out[c, i] = sum_j v[c, j] attn[i, j]
    y = gamma * out + x

Sharding: B=4 batches x 2 query-halves -> 8 cores. Each core computes the
full softmax rows for its 2048 queries against all 4096 keys of its batch.
Host rotates x columns per core so the core's query half is always columns
0:2048 (softmax and the PV contraction are invariant to key/value column
order, as long as K and V use the same order).

Device-side layout tricks:
  - scores computed transposed (sT[j, i]) so that exp output tiles feed the
    PV matmul as the stationary operand without any transposes.
  - V projected directly in transposed layout vT[j, c] (lhsT = x block),
    with an extra ones-column so the PV matmul also produces the softmax
    denominator sum_j exp as output column 256 (per-partition = per-query).
  - epilogue is a single fused per-partition op:
        y_T[i, c] = (out[i, c] * gamma/sum_i) + x_T[i, c]
    written transposed; the host transposes back (pure layout).
"""

import os
import numpy as np

P = 128
B = 4
C = 256
CQ = 32
H = W = 64
N = H * W            # 4096 keys per batch
NH = N // 2          # 2048 queries per core
NCB = C // P         # 2 channel blocks
ST = 512             # query supertile
NST = NH // ST       # 4
JB = N // P          # 32 key blocks

_PROG = None         # cached (nc,) build
LAST_RESULT = None   # BassKernelResults of the last run (for test harness)


def _build_program():
    import concourse.mybir as mybir
    import concourse.tile as tile
    from concourse import bacc
    from concourse.bass import ds

    fp32 = mybir.dt.float32
    bf16 = mybir.dt.bfloat16

    nc = bacc.Bacc(None, target_bir_lowering=False, debug=False)

    x_d = nc.declare_dram_parameter("x", [C, N], fp32, isOutput=False)
    xqT_d = nc.declare_dram_parameter("xqT", [NH, C], fp32, isOutput=False)
    wq_d = nc.declare_dram_parameter("wqT", [C, CQ], bf16, isOutput=False)
    wk_d = nc.declare_dram_parameter("wkT", [C, CQ], bf16, isOutput=False)
    wv_d = nc.declare_dram_parameter("wvT", [C, C], bf16, isOutput=False)
    bq_d = nc.declare_dram_parameter("bq", [CQ, 1], fp32, isOutput=False)
    bk_d = nc.declare_dram_parameter("bk", [CQ, 1], fp32, isOutput=False)
    bv_d = nc.declare_dram_parameter("bv_bc", [P, C], fp32, isOutput=False)
    gm_d = nc.declare_dram_parameter("gamma_bc", [P, 1], fp32, isOutput=False)
    yT_d = nc.declare_dram_parameter("yT", [NH, C], fp32, isOutput=True)

    with tile.TileContext(nc) as tc:
        with (
            tc.tile_pool(name="singles", bufs=1) as singles,
            tc.tile_pool(name="epool", bufs=20) as epool,
            tc.tile_pool(name="stpool", bufs=4) as stpool,
            tc.tile_pool(name="ivpool", bufs=4) as ivpool,
            tc.tile_pool(name="pp_mm", bufs=3, space="PSUM") as pp_mm,
            tc.tile_pool(name="pp_out", bufs=2, space="PSUM") as pp_out,
        ):
            # ---- persistent SBUF tensors ----
            x_sb = singles.tile([P, NCB, N], fp32)
            xb_sb = singles.tile([P, NCB, N], bf16)   # bf16 cast of x for proj
            xqT_sb = singles.tile([P, NH // P, C], fp32)
            wq_sb = singles.tile([P, NCB, CQ], bf16)
            wk_sb = singles.tile([P, NCB, CQ], bf16)
            wv_sb = singles.tile([P, NCB, C], bf16)
            bq_sb = singles.tile([CQ, 1], fp32)
            bk_sb = singles.tile([CQ, 1], fp32)
            bv_sb = singles.tile([P, C], fp32)
            gm_sb = singles.tile([P, 1], fp32)
            kf_sb = singles.tile([P, N], bf16)       # rows 32.. stay zero
            q_sb = singles.tile([P, NH], bf16)       # rows 32.. stay zero
            vT_sb = singles.tile([P, JB, C + 1], bf16)  # col C is all-ones

            # ---- input DMAs (split x into chunks to spread DMA queues) ----
            for cb in range(NCB):
                for hc in range(4):
                    sl = ds(hc * 1024, 1024)
                    nc.sync.dma_start(
                        out=x_sb[:, cb, sl], in_=x_d[cb * P:(cb + 1) * P, sl]
                    )
                    # bf16 cast on gpsimd (idle engine) for the projections
                    nc.gpsimd.tensor_copy(out=xb_sb[:, cb, sl], in_=x_sb[:, cb, sl])
            nc.sync.dma_start(
                out=xqT_sb[:], in_=xqT_d.rearrange("(o p) c -> p o c", p=P)
            )
            nc.sync.dma_start(out=wq_sb[:], in_=wq_d.rearrange("(o p) m -> p o m", p=P))
            nc.sync.dma_start(out=wk_sb[:], in_=wk_d.rearrange("(o p) m -> p o m", p=P))
            nc.sync.dma_start(out=wv_sb[:], in_=wv_d.rearrange("(o p) m -> p o m", p=P))
            nc.sync.dma_start(out=bq_sb[:], in_=bq_d[:])
            nc.sync.dma_start(out=bk_sb[:], in_=bk_d[:])
            nc.sync.dma_start(out=bv_sb[:], in_=bv_d[:])
            nc.sync.dma_start(out=gm_sb[:], in_=gm_d[:])

            # zero the padding rows (contraction runs over all 128 partitions)
            nc.vector.memset(kf_sb[:], 0.0)
            nc.vector.memset(q_sb[:], 0.0)
            nc.vector.memset(vT_sb[:, :, C:C + 1], 1.0)

            # ---- K projection: kf[d, j] = Wk @ x + bk  (bf16, d on partitions)
            for t in range(N // ST):
                ps = pp_mm.tile([CQ, ST], fp32, tag="mm")
                nc.tensor.matmul(
                    ps, wk_sb[:, 0], xb_sb[:, 0, ds(t * ST, ST)], start=True, stop=False
                )
                nc.tensor.matmul(
                    ps, wk_sb[:, 1], xb_sb[:, 1, ds(t * ST, ST)], start=False, stop=True
                )
                nc.vector.tensor_scalar_add(kf_sb[0:CQ, ds(t * ST, ST)], ps, bk_sb)

            # ---- Q projection (only our half: columns 0:NH) ----
            for t in range(NH // ST):
                ps = pp_mm.tile([CQ, ST], fp32, tag="mm")
                nc.tensor.matmul(
                    ps, wq_sb[:, 0], xb_sb[:, 0, ds(t * ST, ST)], start=True, stop=False
                )
                nc.tensor.matmul(
                    ps, wq_sb[:, 1], xb_sb[:, 1, ds(t * ST, ST)], start=False, stop=True
                )
                nc.vector.tensor_scalar_add(q_sb[0:CQ, ds(t * ST, ST)], ps, bq_sb)

            # ---- V projection, transposed: vT[j, c] = sum_c' x[c', j] WvT[c', c]
            for j in range(JB):
                ps = pp_mm.tile([P, C], fp32, tag="mm")
                nc.tensor.matmul(
                    ps, xb_sb[:, 0, ds(j * P, P)], wv_sb[:, 0], start=True, stop=False
                )
                nc.tensor.matmul(
                    ps, xb_sb[:, 1, ds(j * P, P)], wv_sb[:, 1], start=False, stop=True
                )
                nc.vector.tensor_add(vT_sb[:, j, 0:C], ps, bv_sb)

            # ---- attention over query supertiles ----
            # scores for 2 key-blocks at a time -> one big exp -> e persists for
            # the supertile; PV runs one query-block (ib) at a time so the out
            # PSUM pool needs only 2 banks, freeing 6 banks for score tiles.
            for st_i in range(NST):
                q_sl = q_sb[:, ds(st_i * ST, ST)]
                es = []
                for g in range(JB // 2):
                    sps = pp_mm.tile([P, 2, ST], fp32, tag="mm")
                    for u in range(2):
                        j = 2 * g + u
                        nc.tensor.matmul(
                            sps[:, u], kf_sb[:, ds(j * P, P)], q_sl,
                            start=True, stop=True,
                        )
                    e = epool.tile([P, 2, ST], bf16, name=f"e_{st_i}_{g}", tag="e")
                    nc.scalar.activation(
                        e, sps, mybir.ActivationFunctionType.Exp
                    )
                    es.append(e)
                for ib in range(4):
                    out_ps = pp_out.tile(
                        [P, C + 1], fp32, tag="out", name=f"out_{st_i}_{ib}"
                    )
                    for j in range(JB):
                        nc.tensor.matmul(
                            out_ps,
                            es[j // 2][:, j % 2, ds(ib * P, P)],
                            vT_sb[:, j, :],
                            start=(j == 0), stop=(j == JB - 1),
                        )
                    # epilogue: per-partition normalize + gamma + residual
                    row = st_i * 4 + ib
                    inv = ivpool.tile([P, 1], fp32)
                    nc.vector.reciprocal(inv, out_ps[:, C:C + 1])
                    nc.vector.tensor_scalar_mul(inv, inv, gm_sb)
                    stg = stpool.tile([P, C], fp32)
                    nc.vector.scalar_tensor_tensor(
                        stg, out_ps[:, 0:C], inv, xqT_sb[:, row, :],
                        op0=mybir.AluOpType.mult,
                        op1=mybir.AluOpType.add,
                    )
                    nc.sync.dma_start(out=yT_d[ds(row * P, P), :], in_=stg)

    return nc


def _get_program():
    global _PROG
    if _PROG is None:
        _PROG = _build_program()
        if not _PROG.is_finalized():
            _PROG.finalize()
    return _PROG


def kernel(x, Wq, bq, Wk, bk, Wv, bv, gamma):
    global LAST_RESULT
    from concourse.bass_utils import run_bass_kernel_spmd

    x = np.ascontiguousarray(np.asarray(x, dtype=np.float32))
    Wq = np.asarray(Wq, dtype=np.float32)
    bq = np.asarray(bq, dtype=np.float32)
    Wk = np.asarray(Wk, dtype=np.float32)
    bk = np.asarray(bk, dtype=np.float32)
    Wv = np.asarray(Wv, dtype=np.float32)
    bv = np.asarray(bv, dtype=np.float32)
    gamma = np.asarray(gamma, dtype=np.float32)

    import ml_dtypes
    wqT = np.ascontiguousarray(Wq.T.astype(ml_dtypes.bfloat16))
    wkT = np.ascontiguousarray(Wk.T.astype(ml_dtypes.bfloat16))
    wvT = np.ascontiguousarray(Wv.T.astype(ml_dtypes.bfloat16))
    bq2 = np.ascontiguousarray(bq[:, None])
    bk2 = np.ascontiguousarray(bk[:, None])
    bv_bc = np.ascontiguousarray(np.broadcast_to(bv[None, :], (P, C)))
    gm_bc = np.full((P, 1), float(gamma.reshape(-1)[0]), dtype=np.float32)

    xf = x.reshape(B, C, N)
    in_maps = []
    for core in range(8):
        b, h = core // 2, core % 2
        xb = xf[b]
        if h == 0:
            x_roll = np.ascontiguousarray(xb)
        else:
            x_roll = np.ascontiguousarray(
                np.concatenate([xb[:, NH:], xb[:, :NH]], axis=1)
            )
        xqT = np.ascontiguousarray(x_roll[:, :NH].T)
        in_maps.append({
            "x": x_roll,
            "xqT": xqT,
            "wqT": wqT,
            "wkT": wkT,
            "wvT": wvT,
            "bq": bq2,
            "bk": bk2,
            "bv_bc": bv_bc,
            "gamma_bc": gm_bc,
        })

    nc = _get_program()
    res = run_bass_kernel_spmd(
        nc, in_maps, core_ids=list(range(8)),
        trace=bool(os.environ.get("BASS_TRACE")),
    )
    LAST_RESULT = res

    out = np.empty((B, C, N), dtype=np.float32)
    for core in range(8):
        b, h = core // 2, core % 2
        yT = res.results[core]["yT"]
        out[b][:, h * NH:(h + 1) * NH] = yT.T
    return out.reshape(B, C, H, W)


# revision 9
# speedup vs baseline: 1.4815x; 1.1250x over previous
"""Position-attention kernel for Trainium2 (8 NeuronCores, SPMD).

Reference computation (per batch b):
    q = Wq @ x + bq        [32, 4096]
    k = Wk @ x + bk        [32, 4096]
    v = Wv @ x + bv        [256, 4096]
    attn = softmax_j(q_i . k_j)           [4096, 4096]
    out[c, i] = sum_j v[c, j] attn[i, j]
    y = gamma * out + x

Sharding: B=4 batches x 2 query-halves -> 8 cores. Each core computes the
full softmax rows for its 2048 queries against all 4096 keys of its batch.
Host rotates x columns per core so the core's query half is always columns
0:2048 (softmax and the PV contraction are invariant to key/value column
order, as long as K and V use the same order).

Device-side structure (per core):
  - projections in bf16 (x pre-cast on host; weights pre-packed on host).
  - scores computed transposed (sT[j, i]) in PSUM, 2 key-blocks at a time
    packed into PE row-groups 0/32 via tile_position (K=32 contractions run
    concurrently); kf is stored packed ([d + 32*r] rows), q replicated 4x.
  - one exp (ACT) per 2 key-blocks: PSUM [128, 2, 512] -> SBUF bf16; these
    e-tiles persist for the whole query supertile.
  - PV: out[i, c] = sum_j e[j, i] * vT[j, c] with e-blocks as the stationary
    operand; vT carries an extra all-ones column so column 256 of the
    output is the softmax denominator (per-partition = per-query).
  - epilogue: y_T[i, :] = out[i, :] * (gamma / sum_i) + (x_T[i, :] +
    gamma * bv)  -- the bv term works because sum_j attn = 1; it is folded
    into the precomputed xpb tile. Output written transposed; host
    transposes back (pure layout).
"""

import os
import numpy as np

P = 128
B = 4
C = 256
CQ = 32
H = W = 64
N = H * W            # 4096 keys per batch
NH = N // 2          # 2048 queries per core
NCB = C // P         # 2 channel blocks
ST = 512             # query supertile
NST = NH // ST       # 4
JB = N // P          # 32 key blocks
NG = JB // 2         # 16 score groups (2 key blocks each)

_PROG = None         # cached build
LAST_RESULT = None   # BassKernelResults of the last run (for test harness)


def _build_program():
    import concourse.mybir as mybir
    import concourse.tile as tile
    from concourse import bacc
    from concourse.bass import ds

    fp32 = mybir.dt.float32
    bf16 = mybir.dt.bfloat16

    nc = bacc.Bacc(None, target_bir_lowering=False, debug=False)

    xb_d = nc.declare_dram_parameter("xb", [C, N], bf16, isOutput=False)
    xqT_d = nc.declare_dram_parameter("xqT", [NH, C], fp32, isOutput=False)
    wq_d = nc.declare_dram_parameter("wq_rep", [C, P], bf16, isOutput=False)
    wk_d = nc.declare_dram_parameter("wk_pack", [C, 2, P], bf16, isOutput=False)
    wv_d = nc.declare_dram_parameter("wvT", [C, C], bf16, isOutput=False)
    bq_d = nc.declare_dram_parameter("bq_rep", [P, 1], fp32, isOutput=False)
    bk_d = nc.declare_dram_parameter("bk_pack", [P, 1], fp32, isOutput=False)
    bv_d = nc.declare_dram_parameter("bv_bc", [P, C], fp32, isOutput=False)
    gm_d = nc.declare_dram_parameter("gamma_bc", [P, 1], fp32, isOutput=False)
    yT_d = nc.declare_dram_parameter("yT", [NH, C], fp32, isOutput=True)

    with tile.TileContext(nc) as tc:
        with (
            tc.tile_pool(name="singles", bufs=1) as singles,
            tc.tile_pool(name="epool", bufs=20) as epool,
            tc.tile_pool(name="stpool", bufs=4) as stpool,
            tc.tile_pool(name="ivpool", bufs=4) as ivpool,
            tc.tile_pool(name="pp_mm", bufs=3, space="PSUM") as pp_mm,
            tc.tile_pool(name="pp_out", bufs=2, space="PSUM") as pp_out,
        ):
            # ---- persistent SBUF tensors ----
            xb_sb = singles.tile([P, NCB, N], bf16)
            xqT_sb = singles.tile([P, NH // P, C], fp32)
            xpb_sb = singles.tile([P, NH // P, C], fp32)  # xT + gamma*bv
            wq_sb = singles.tile([P, NCB, P], bf16)
            wk_sb = singles.tile([P, NCB, 2, P], bf16)
            wv_sb = singles.tile([P, NCB, C], bf16)
            bq_sb = singles.tile([P, 1], fp32)
            bk_sb = singles.tile([P, 1], fp32)
            bv_sb = singles.tile([P, C], fp32)
            gm_sb = singles.tile([P, 1], fp32)
            kf_sb = singles.tile([P, NG, P], bf16)   # packed: row 32r+d, grp g
            q_sb = singles.tile([P, NH], bf16)       # q replicated in 4 groups
            vT_sb = singles.tile([P, JB, C + 1], bf16)  # col C is all-ones

            # ---- input DMAs (weights first; xqT last - only needed at end)
            nc.sync.dma_start(out=wq_sb[:], in_=wq_d.rearrange("(o p) m -> p o m", p=P))
            nc.sync.dma_start(
                out=wk_sb[:], in_=wk_d.rearrange("(o p) r m -> p o r m", p=P)
            )
            nc.sync.dma_start(out=wv_sb[:], in_=wv_d.rearrange("(o p) m -> p o m", p=P))
            nc.sync.dma_start(out=bq_sb[:], in_=bq_d[:])
            nc.sync.dma_start(out=bk_sb[:], in_=bk_d[:])
            nc.sync.dma_start(out=bv_sb[:], in_=bv_d[:])
            nc.sync.dma_start(out=gm_sb[:], in_=gm_d[:])
            for cb in range(NCB):
                for hc in range(4):
                    sl = ds(hc * 1024, 1024)
                    nc.sync.dma_start(
                        out=xb_sb[:, cb, sl], in_=xb_d[cb * P:(cb + 1) * P, sl]
                    )
            nc.sync.dma_start(
                out=xqT_sb[:], in_=xqT_d.rearrange("(o p) c -> p o c", p=P)
            )

            nc.vector.memset(vT_sb[:, :, C:C + 1], 1.0)

            # ---- K projection into packed layout ----
            # kf_sb[32r + d, g, :] = (Wk @ x + bk)[d, (2g + r)*128 : ...]
            # wk variant r has WkT at column offset 32r (zeros elsewhere), so
            # the four accumulating matmuls write disjoint row blocks.
            for g in range(NG):
                kp = pp_mm.tile([P, P], fp32, tag="mm")
                for i, (r, cb) in enumerate(
                    [(r, cb) for r in range(2) for cb in range(NCB)]
                ):
                    nc.tensor.matmul(
                        kp, wk_sb[:, cb, r], xb_sb[:, cb, ds((2 * g + r) * P, P)],
                        start=(i == 0), stop=(i == 3),
                    )
                nc.vector.tensor_scalar_add(kf_sb[:, g, :], kp, bk_sb)

            # ---- Q projection, replicated across the 4 row groups ----
            for t in range(NH // ST):
                qp = pp_mm.tile([P, ST], fp32, tag="mm")
                nc.tensor.matmul(
                    qp, wq_sb[:, 0], xb_sb[:, 0, ds(t * ST, ST)],
                    start=True, stop=False,
                )
                nc.tensor.matmul(
                    qp, wq_sb[:, 1], xb_sb[:, 1, ds(t * ST, ST)],
                    start=False, stop=True,
                )
                nc.vector.tensor_scalar_add(q_sb[:, ds(t * ST, ST)], qp, bq_sb)

            # ---- V projection, transposed (no bias: bv is folded into xpb)
            for j in range(JB):
                vp = pp_mm.tile([P, C], fp32, tag="mm")
                nc.tensor.matmul(
                    vp, xb_sb[:, 0, ds(j * P, P)], wv_sb[:, 0], start=True, stop=False
                )
                nc.tensor.matmul(
                    vp, xb_sb[:, 1, ds(j * P, P)], wv_sb[:, 1], start=False, stop=True
                )
                # split copies between DVE and ACT so neither serializes PE
                if j % 2 == 0:
                    nc.vector.tensor_copy(vT_sb[:, j, 0:C], vp)
                else:
                    nc.scalar.copy(vT_sb[:, j, 0:C], vp)

            # ---- xpb = xT + gamma*bv  (valid because sum_j attn == 1) ----
            for rw in range(NH // P):
                nc.vector.scalar_tensor_tensor(
                    xpb_sb[:, rw, :], bv_sb, gm_sb, xqT_sb[:, rw, :],
                    op0=mybir.AluOpType.mult,
                    op1=mybir.AluOpType.add,
                )

            # ---- attention over query supertiles ----
            for st_i in range(NST):
                es = []
                for g in range(NG):
                    sps = pp_mm.tile([P, 2, ST], fp32, tag="mm")
                    for r in range(2):
                        nc.tensor.matmul(
                            sps[:, r],
                            kf_sb[32 * r:32 * (r + 1), g, :],
                            q_sb[32 * r:32 * (r + 1), ds(st_i * ST, ST)],
                            start=True, stop=True,
                            tile_position=(32 * r, 0),
                        )
                    e = epool.tile([P, 2, ST], bf16, name=f"e_{st_i}_{g}", tag="e")
                    nc.scalar.activation(e, sps, mybir.ActivationFunctionType.Exp)
                    es.append(e)
                for ib in range(4):
                    out_ps = pp_out.tile(
                        [P, C + 1], fp32, tag="out", name=f"out_{st_i}_{ib}"
                    )
                    for j in range(JB):
                        nc.tensor.matmul(
                            out_ps,
                            es[j // 2][:, j % 2, ds(ib * P, P)],
                            vT_sb[:, j, :],
                            start=(j == 0), stop=(j == JB - 1),
                        )
                    # epilogue: per-partition normalize + gamma + residual
                    row = st_i * 4 + ib
                    inv = ivpool.tile([P, 1], fp32)
                    nc.vector.reciprocal(inv, out_ps[:, C:C + 1])
                    nc.vector.tensor_scalar_mul(inv, inv, gm_sb)
                    stg = stpool.tile([P, C], fp32)
                    nc.vector.scalar_tensor_tensor(
                        stg, out_ps[:, 0:C], inv, xpb_sb[:, row, :],
                        op0=mybir.AluOpType.mult,
                        op1=mybir.AluOpType.add,
                    )
                    nc.sync.dma_start(out=yT_d[ds(row * P, P), :], in_=stg)

    return nc


def _get_program():
    global _PROG
    if _PROG is None:
        _PROG = _build_program()
        if not _PROG.is_finalized():
            _PROG.finalize()
    return _PROG


def kernel(x, Wq, bq, Wk, bk, Wv, bv, gamma):
    global LAST_RESULT
    import ml_dtypes
    from concourse.bass_utils import run_bass_kernel_spmd

    bf16 = ml_dtypes.bfloat16
    x = np.ascontiguousarray(np.asarray(x, dtype=np.float32))
    Wq = np.asarray(Wq, dtype=np.float32)
    bq = np.asarray(bq, dtype=np.float32)
    Wk = np.asarray(Wk, dtype=np.float32)
    bk = np.asarray(bk, dtype=np.float32)
    Wv = np.asarray(Wv, dtype=np.float32)
    bv = np.asarray(bv, dtype=np.float32)
    gamma = np.asarray(gamma, dtype=np.float32)

    # wq replicated into all four 32-row groups of the PE array
    wq_rep = np.zeros((C, P), dtype=np.float32)
    for r in range(4):
        wq_rep[:, 32 * r:32 * (r + 1)] = Wq.T
    # wk variant r carries WkT at column offset 32r (r = 0, 1)
    wk_pack = np.zeros((C, 2, P), dtype=np.float32)
    for r in range(2):
        wk_pack[:, r, 32 * r:32 * (r + 1)] = Wk.T
    bq_rep = np.tile(bq, 4)[:, None].astype(np.float32)
    bk_pack = np.zeros((P, 1), dtype=np.float32)
    bk_pack[0:32, 0] = bk
    bk_pack[32:64, 0] = bk
    bv_bc = np.ascontiguousarray(np.broadcast_to(bv[None, :], (P, C))).astype(
        np.float32
    )
    gm_bc = np.full((P, 1), float(gamma.reshape(-1)[0]), dtype=np.float32)

    wq_rep = np.ascontiguousarray(wq_rep.astype(bf16))
    wk_pack = np.ascontiguousarray(wk_pack.astype(bf16))
    wvT = np.ascontiguousarray(Wv.T.astype(bf16))

    xf = x.reshape(B, C, N)
    in_maps = []
    for core in range(8):
        b, h = core // 2, core % 2
        xb = xf[b]
        if h == 0:
            x_roll = xb
        else:
            x_roll = np.concatenate([xb[:, NH:], xb[:, :NH]], axis=1)
        xqT = np.ascontiguousarray(x_roll[:, :NH].T)
        in_maps.append({
            "xb": np.ascontiguousarray(x_roll.astype(bf16)),
            "xqT": xqT,
            "wq_rep": wq_rep,
            "wk_pack": wk_pack,
            "wvT": wvT,
            "bq_rep": bq_rep,
            "bk_pack": bk_pack,
            "bv_bc": bv_bc,
            "gamma_bc": gm_bc,
        })

    nc = _get_program()
    res = run_bass_kernel_spmd(
        nc, in_maps, core_ids=list(range(8)),
        trace=bool(os.environ.get("BASS_TRACE")),
    )
    LAST_RESULT = res

    out = np.empty((B, C, N), dtype=np.float32)
    for core in range(8):
        b, h = core // 2, core % 2
        yT = res.results[core]["yT"]
        out[b][:, h * NH:(h + 1) * NH] = yT.T
    return out.reshape(B, C, H, W)
